# revision 4
# baseline (speedup 1.0000x reference)
"""MiniRocketFeaturesPlus Trainium2 kernel.

Strategy (data-parallel over batch, 4 samples per core on 8 cores):
  - Grouped dilated conv + channel combination folded into one matmul per
    512-col chunk: C[j, l] = sum_{k=(t,c)} W2[k, j] * Xshift[k, l], K=81.
  - Per dilation d: a shifted slab Xmat[81, 4*(2048+p)+512] is DMA-gathered
    straight from a padded DRAM copy of x (affine AP; tap shift = d*(t-4)).
  - PSUM tile [128, 2048] per (dilation, sample): rows 0..41 = full-window
    half kernels, 64..105 = cropped-half kernels computed over the shifted
    window [p, p+2048) (tail beyond the valid region multiplied against a
    zeroed rhs slab so it lands as exact zeros).  M is zero-padded to 64.
  - PPV reductions as fused compute+row-reduce instructions reading PSUM
    directly: counts (is_gt / Sign), relu-sums (max / Relu), plain sums.
    Statically load-balanced across the Vector and Scalar engines.
  - Per-row accumulator columns are the only device output; the cheap
    affine decode (tail corrections, a = N/L, b = R / max(2R - S + L*t,
    1e-8)) and final feature concatenation happen on the host.
"""

import math
import sys

import numpy as np

sys.path.insert(0, "/opt/trn_rl_repo")

C_IN, SEQ_LEN = 9, 2048
KERNEL_SIZE = 9
NUM_KERNELS = 84
B = 32
N_CORES = 8
B_CORE = B // N_CORES
PAD = 1020  # max padding over all dilations
LP = SEQ_LEN + 2 * PAD  # 4088
ZERO_W = 512  # width of the zeroed rhs region
NEG_BIG = -1.0e30

DVE_COST = 2.35  # us per [128, 2048] fused pass (measured)
ACT_COST = 2.20


def _config():
    nf_total = 10000 // 2 // NUM_KERNELS * NUM_KERNELS
    nfpk = nf_total // NUM_KERNELS
    true_max = min(nfpk, 32)
    multiplier = nfpk / true_max
    max_exp = np.log2((SEQ_LEN - 1) / (KERNEL_SIZE - 1))
    dilations, counts = np.unique(
        np.logspace(0, max_exp, true_max, base=2).astype(np.int32),
        return_counts=True)
    nfpd = (counts * multiplier).astype(np.int32)
    rem = nfpk - nfpd.sum()
    i = 0
    while rem > 0:
        nfpd[i] += 1
        rem -= 1
        i = (i + 1) % len(nfpd)
    paddings = [(KERNEL_SIZE - 1) * int(d) // 2 for d in dilations]
    return [int(d) for d in dilations], paddings, [int(n) for n in nfpd]


DILS, PADS, NFPD = _config()
ND = len(DILS)


class Plan:
    """Static schedule: pass columns, engine assignment, decode constants."""

    def __init__(self):
        self.passes = []  # list of dicts, one per (dil, type, f) pass family
        dve_load = 0.0
        act_load = 0.0
        for i, (d, p, nf) in enumerate(zip(DILS, PADS, NFPD)):
            fams = [("cnt", f) for f in range(nf)] + \
                   [("rel", f) for f in range(nf)] + [("sum", 0)]
            for typ, f in fams:
                if dve_load <= act_load:
                    eng, cost = "dve", DVE_COST
                else:
                    eng, cost = "act", ACT_COST
                if eng == "dve":
                    dve_load += 4 * cost
                else:
                    act_load += 4 * cost
                self.passes.append(dict(dil=i, typ=typ, f=f, eng=eng))
        # per-engine accumulator column indexing: each pass family uses
        # B_CORE consecutive columns (one per sample tile)
        ncol = {"dve": 0, "act": 0}
        thrcol = 0
        for pa in self.passes:
            pa["acc0"] = ncol[pa["eng"]]
            ncol[pa["eng"]] += B_CORE
            pa["thr"] = thrcol
            thrcol += 1
        self.ncol_dve = ncol["dve"]
        self.ncol_act = ncol["act"]
        self.nthr = thrcol
        self.off = np.concatenate([[0], np.cumsum(NFPD)]).astype(int)

    def halves(self, i):
        p1 = i % 2
        h0 = list(range(p1, NUM_KERNELS, 2))
        h1 = list(range(1 - p1, NUM_KERNELS, 2))
        return h0, h1

    def build_thresholds(self, biases):
        """thr[128, nthr] per-partition scalar operand for every pass family."""
        thr = np.zeros((128, self.nthr), np.float32)
        for pa in self.passes:
            i, typ, f, eng = pa["dil"], pa["typ"], pa["f"], pa["eng"]
            h0, h1 = self.halves(i)
            col = np.zeros(128, np.float32)
            if typ == "sum":
                col[:] = NEG_BIG if eng == "dve" else 0.0
            else:
                t0 = biases[h0, self.off[i] + f].astype(np.float32)
                t1 = biases[h1, self.off[i] + f].astype(np.float32)
                if eng == "dve":
                    col[0:42], col[64:106] = t0, t1
                elif typ == "cnt":
                    # ACT Sign pass: bias = -t', t' nudged off any fp32 value
                    # equal to a representable C so sign() never hits 0.
                    # C is fp32 here; nudge by ~1ulp of the threshold scale.
                    tp0 = np.nextafter(t0, np.float32(np.inf)).astype(np.float32)
                    tp1 = np.nextafter(t1, np.float32(np.inf)).astype(np.float32)
                    col[0:42], col[64:106] = -tp0, -tp1
                else:
                    col[0:42], col[64:106] = -t0, -t1
            thr[:, pa["thr"]] = col
        return thr

    def decode(self, acc_dve, acc_act, biases):
        """acc_*: [ncores, 128, ncol_*] fp32 -> output [B, 9912] fp32."""
        biases = np.asarray(biases, np.float64)
        nunits = 2  # h0 rows 0..41, h1 rows 64..105
        # gather per (core, b, dil): N[84, nf], R[84, nf], S[84]
        out = np.zeros((B, sum(168 * nf for nf in NFPD)), np.float64)
        # feature block offsets per dilation
        blk_off = np.concatenate([[0], np.cumsum([168 * nf for nf in NFPD])]).astype(int)
        for pa in self.passes:
            pa["_acc"] = (acc_dve if pa["eng"] == "dve" else acc_act)[
                :, :, pa["acc0"]:pa["acc0"] + B_CORE].astype(np.float64)
        N = {}
        R = {}
        S = {}
        for pa in self.passes:
            i, typ, f, eng = pa["dil"], pa["typ"], pa["f"], pa["eng"]
            d, p, nf = DILS[i], PADS[i], NFPD[i]
            h0, h1 = self.halves(i)
            V = SEQ_LEN - 2 * p          # valid length for h1
            T2 = SEQ_LEN - V             # zero-tail length for h1
            a = pa["_acc"]               # [ncores, 128, B_CORE]
            r0 = a[:, 0:42, :]           # h0 rows
            r1 = a[:, 64:106, :]         # h1 rows
            t0 = biases[h0, self.off[i] + f][None, :, None]
            t1 = biases[h1, self.off[i] + f][None, :, None]
            if typ == "cnt":
                if eng == "dve":
                    n0 = r0
                    n1 = r1 - T2 * (t1 < 0)
                else:
                    n0 = (r0 + SEQ_LEN) / 2
                    sv = r1 + T2 * np.sign(t1)
                    n1 = (sv + V) / 2
                N[(i, f)] = (n0, n1)
            elif typ == "rel":
                if eng == "dve":
                    rr0 = r0 - SEQ_LEN * t0
                    rr1 = r1 - T2 * np.maximum(t1, 0) - V * t1
                else:
                    rr0 = r0
                    rr1 = r1 - T2 * np.maximum(-t1, 0)
                R[(i, f)] = (rr0, rr1)
            else:
                S[i] = (r0, r1)
        for i, (d, p, nf) in enumerate(zip(DILS, PADS, NFPD)):
            h0, h1 = self.halves(i)
            V = SEQ_LEN - 2 * p
            s0, s1 = S[i]
            for f in range(nf):
                t0 = biases[h0, self.off[i] + f][None, :, None]
                t1 = biases[h1, self.off[i] + f][None, :, None]
                n0, n1 = N[(i, f)]
                r0, r1 = R[(i, f)]
                a0 = n0 / SEQ_LEN
                a1 = n1 / V
                d0 = s0 - SEQ_LEN * t0
                d1 = s1 - V * t1
                b0 = r0 / np.maximum(2 * r0 - d0, 1e-8)
                b1 = r1 / np.maximum(2 * r1 - d1, 1e-8)
                # output layout per dil: [a_h0(42,nf) | b_h0 | a_h1 | b_h1],
                # feature index within block = jj*nf + f
                base = blk_off[i]
                for core in range(N_CORES):
                    for bb in range(B_CORE):
                        gb = core * B_CORE + bb
                        out[gb, base + f + nf * np.arange(42)] = a0[core, :, bb]
                        out[gb, base + 42 * nf + f + nf * np.arange(42)] = b0[core, :, bb]
                        out[gb, base + 84 * nf + f + nf * np.arange(42)] = a1[core, :, bb]
                        out[gb, base + 126 * nf + f + nf * np.arange(42)] = b1[core, :, bb]
        return out.astype(np.float32)


PLAN = Plan()
_NC_CACHE = {}


def _build_nc():
    import concourse.bacc as bacc
    import concourse.tile as tile
    from concourse import mybir

    nc = bacc.Bacc()
    x_pad = nc.dram_tensor("x_pad", [C_IN, B_CORE, LP], mybir.dt.float32,
                           kind="ExternalInput")
    w2 = nc.dram_tensor("w2", [ND, 2, 81, 64], mybir.dt.float32,
                        kind="ExternalInput")
    thr_in = nc.dram_tensor("thr_in", [128, PLAN.nthr], mybir.dt.float32,
                            kind="ExternalInput")
    acc_dve_out = nc.dram_tensor("acc_dve", [128, PLAN.ncol_dve],
                                 mybir.dt.float32, kind="ExternalOutput")
    acc_act_out = nc.dram_tensor("acc_act", [128, PLAN.ncol_act],
                                 mybir.dt.float32, kind="ExternalOutput")

    WMAX = SEQ_LEN                      # per-sample slab width
    XCOLS = B_CORE * WMAX + ZERO_W

    with tile.TileContext(nc) as tc:
        with tc.tile_pool(name="sb", bufs=1) as sb, \
             tc.tile_pool(name="ps", bufs=2, space="PSUM") as ps:
            w2_sb = sb.tile([81, ND, 2, 64], mybir.dt.float32, tag="w2")
            thr = sb.tile([128, PLAN.nthr], mybir.dt.float32, tag="thr")
            acc_dve = sb.tile([128, PLAN.ncol_dve], mybir.dt.float32, tag="accd")
            acc_act = sb.tile([128, PLAN.ncol_act], mybir.dt.float32, tag="acca")
            junk_d = sb.tile([128, SEQ_LEN], mybir.dt.bfloat16, tag="junkd")
            junk_a = sb.tile([128, SEQ_LEN], mybir.dt.bfloat16, tag="junka")
            xmat = []
            for k in range(2):
                xmk = sb.tile([81, XCOLS], mybir.dt.float32, tag=f"xmat{k}")
                xmat.append(xmk)

            nc.sync.dma_start(out=w2_sb,
                              in_=w2[:, :, :, :].transpose([2, 0, 1, 3]))
            nc.sync.dma_start(out=thr, in_=thr_in[:, :])
            for k in range(2):
                nc.vector.memset(xmat[k][:, B_CORE * WMAX:], 0.0)

            for i, (d, p, nf) in enumerate(zip(DILS, PADS, NFPD)):
                W = SEQ_LEN              # per-sample slab width
                xm = xmat[i % 2]
                # gather the shifted slab straight from DRAM:
                # xm[(t*9+c), b*W + l] = x_pad[c, b, PAD - 4d + d*t + l]
                import concourse.bass as bass
                full = x_pad[:, :, :]
                for t in range(9):
                    src_ap = bass.AP(
                        tensor=full.tensor,
                        offset=PAD - 4 * d + d * t,
                        ap=[[B_CORE * LP, 9], [LP, B_CORE], [1, W]],
                    )
                    dst_ap = bass.AP(
                        tensor=xm.tensor,
                        offset=xm.offset + t * 9 * XCOLS,
                        ap=[[XCOLS, 9], [W, B_CORE], [1, W]],
                    )
                    nc.sync.dma_start(out=dst_ap, in_=src_ap)

                lhs0 = w2_sb[:, i, 0, :]
                lhs1 = w2_sb[:, i, 1, :]
                V = SEQ_LEN - 2 * p
                kv = V // 512
                for bb in range(B_CORE):
                    pt = ps.tile([128, SEQ_LEN], mybir.dt.float32, tag="pt")
                    x0 = bb * W
                    # h0: full window [0, 2048)
                    for k in range(4):
                        nc.tensor.matmul(pt[0:64, 512 * k:512 * (k + 1)],
                                         lhs0, xm[:, x0 + 512 * k: x0 + 512 * (k + 1)],
                                         start=True, stop=True)
                    # h1: window [p, p+2048); tail beyond valid uses zero rhs
                    for k in range(kv):
                        nc.tensor.matmul(pt[64:128, 512 * k:512 * (k + 1)],
                                         lhs1,
                                         xm[:, x0 + p + 512 * k: x0 + p + 512 * (k + 1)],
                                         start=True, stop=True)
                    rem = V - 512 * kv
                    if rem > 0:
                        nc.tensor.matmul(pt[64:128, 512 * kv:512 * kv + rem],
                                         lhs1, xm[:, x0 + p + 512 * kv: x0 + 2048 - p],
                                         start=True, stop=True)
                    zoff = B_CORE * WMAX
                    zstart = V
                    while zstart < SEQ_LEN:
                        zlen = min(512, SEQ_LEN - zstart,
                                   512 * (zstart // 512 + 1) - zstart)
                        nc.tensor.matmul(pt[64:128, zstart:zstart + zlen],
                                         lhs1, xm[:, zoff:zoff + zlen],
                                         start=True, stop=True)
                        zstart += zlen

                    # fused reduce passes over the finished tile
                    for pa in PLAN.passes:
                        if pa["dil"] != i:
                            continue
                        tcol = thr[:, pa["thr"]:pa["thr"] + 1]
                        if pa["eng"] == "dve":
                            acol = acc_dve[:, pa["acc0"] + bb: pa["acc0"] + bb + 1]
                            op0 = (mybir.AluOpType.is_gt if pa["typ"] == "cnt"
                                   else mybir.AluOpType.max)
                            nc.vector.tensor_scalar(
                                out=junk_d, in0=pt[:, :], scalar1=tcol,
                                scalar2=None, op0=op0,
                                op1=mybir.AluOpType.add, accum_out=acol)
                        else:
                            acol = acc_act[:, pa["acc0"] + bb: pa["acc0"] + bb + 1]
                            if pa["typ"] == "cnt":
                                nc.scalar.activation(
                                    out=junk_a, in_=pt[:, :],
                                    func=mybir.ActivationFunctionType.Sign,
                                    bias=tcol, scale=1.0, accum_out=acol)
                            elif pa["typ"] == "rel":
                                nc.scalar.activation(
                                    out=junk_a, in_=pt[:, :],
                                    func=mybir.ActivationFunctionType.Relu,
                                    bias=tcol, scale=1.0, accum_out=acol)
                            else:
                                nc.scalar.activation(
                                    out=junk_a, in_=pt[:, :],
                                    func=mybir.ActivationFunctionType.Copy,
                                    bias=0.0, scale=1.0, accum_out=acol)

            nc.sync.dma_start(out=acc_dve_out[:, :], in_=acc_dve)
            nc.sync.dma_start(out=acc_act_out[:, :], in_=acc_act)
    nc.compile()
    return nc


def _host_prep(x, kernels, channel_combinations):
    x = np.asarray(x, np.float32)
    kernels = np.asarray(kernels, np.float32)
    cc = np.asarray(channel_combinations, np.float32)
    x_pad = np.zeros((B, C_IN, LP), np.float32)
    x_pad[:, :, PAD:PAD + SEQ_LEN] = x
    # W2[i, h, t*9+c, jj] = cc[i, c, j] * kernels[c*84+j, 0, t], j = halves
    kern = kernels.reshape(C_IN, NUM_KERNELS, KERNEL_SIZE)  # [c, j, t]
    w2 = np.zeros((ND, 2, 81, 64), np.float32)
    for i in range(ND):
        full = np.einsum("cj,cjt->tcj", cc[i], kern)  # [t, c, j]
        full = full.reshape(81, NUM_KERNELS)
        h0, h1 = PLAN.halves(i)
        w2[i, 0, :, :42] = full[:, h0]
        w2[i, 1, :, :42] = full[:, h1]
    return x_pad, w2


def kernel(x, kernels, channel_combinations, biases, _run_cores=None):
    from concourse.bass_utils import run_bass_kernel_spmd

    x_pad, w2 = _host_prep(x, kernels, channel_combinations)
    thr = PLAN.build_thresholds(np.asarray(biases, np.float32))

    if "nc" not in _NC_CACHE:
        _NC_CACHE["nc"] = _build_nc()
    nc = _NC_CACHE["nc"]

    cores = list(range(N_CORES)) if _run_cores is None else _run_cores
    in_maps = []
    for core in cores:
        xs = x_pad[core * B_CORE:(core + 1) * B_CORE]  # [B_CORE, 9, LP]
        in_maps.append({
            "x_pad": np.ascontiguousarray(xs.transpose(1, 0, 2)),
            "w2": w2,
            "thr_in": thr,
        })
    res = run_bass_kernel_spmd(nc, in_maps, core_ids=cores)
    acc_dve = np.stack([r["acc_dve"] for r in res.results])
    acc_act = np.stack([r["acc_act"] for r in res.results])
    if _run_cores is not None:  # partial run (debug): tile results
        reps = N_CORES // len(cores)
        acc_dve = np.concatenate([acc_dve] * reps)
        acc_act = np.concatenate([acc_act] * reps)
    return PLAN.decode(acc_dve, acc_act, biases)


def sim_accums(x, kernels, channel_combinations, biases):
    """Numpy simulation of the device accumulators (for decode validation)."""
    x_pad, w2 = _host_prep(x, kernels, channel_combinations)
    acc_dve = np.zeros((N_CORES, 128, PLAN.ncol_dve), np.float32)
    acc_act = np.zeros((N_CORES, 128, PLAN.ncol_act), np.float32)
    thr = PLAN.build_thresholds(np.asarray(biases, np.float32))
    for core in range(N_CORES):
        xs = x_pad[core * B_CORE:(core + 1) * B_CORE]  # [b, c, LP]
        for i, (d, p, nf) in enumerate(zip(DILS, PADS, NFPD)):
            W = SEQ_LEN
            V = SEQ_LEN - 2 * p
            # Xslab[k=(t,c), b, l]
            xsl = np.zeros((81, B_CORE, W), np.float32)
            for t in range(9):
                for c in range(9):
                    o = PAD - 4 * d + d * t
                    xsl[t * 9 + c] = xs[:, c, o:o + W]
            for bb in range(B_CORE):
                C = np.zeros((128, SEQ_LEN), np.float32)
                C[0:64] = w2[i, 0].T @ xsl[:, bb, :SEQ_LEN]
                C[64:128, :V] = w2[i, 1].T @ xsl[:, bb, p:SEQ_LEN - p]
                for pa in PLAN.passes:
                    if pa["dil"] != i:
                        continue
                    tc_ = thr[:, pa["thr"]][:, None]
                    if pa["eng"] == "dve":
                        if pa["typ"] == "cnt":
                            v = (C > tc_).sum(1, dtype=np.float32)
                        else:
                            v = np.maximum(C, tc_).sum(1, dtype=np.float64).astype(np.float32)
                        acc_dve[core, :, pa["acc0"] + bb] = v
                    else:
                        if pa["typ"] == "cnt":
                            v = np.sign(C + tc_).sum(1).astype(np.float32)
                        elif pa["typ"] == "rel":
                            v = np.maximum(C + tc_, 0).sum(1, dtype=np.float64).astype(np.float32)
                        else:
                            v = C.sum(1, dtype=np.float64).astype(np.float32)
                        acc_act[core, :, pa["acc0"] + bb] = v
    return acc_dve, acc_act


# revision 7
# speedup vs baseline: 1.0543x; 1.0543x over previous
"""MiniRocketFeaturesPlus Trainium2 kernel.

Strategy (data-parallel over batch, 4 samples per core on 8 cores):
  - Grouped dilated conv + channel combination folded into one matmul per
    512-col chunk: C[j, l] = sum_{k=(t,c)} W2[k, j] * Xshift[k, l], K=81.
  - Per dilation d: a shifted slab Xmat[81, 4*(2048+p)+512] is DMA-gathered
    straight from a padded DRAM copy of x (affine AP; tap shift = d*(t-4)).
  - PSUM tile [128, 2048] per (dilation, sample): rows 0..41 = full-window
    half kernels, 64..105 = cropped-half kernels computed over the shifted
    window [p, p+2048) (tail beyond the valid region multiplied against a
    zeroed rhs slab so it lands as exact zeros).  M is zero-padded to 64.
  - PPV reductions as fused compute+row-reduce instructions reading PSUM
    directly: counts (is_gt / Sign), relu-sums (max / Relu), plain sums.
    Statically load-balanced across the Vector and Scalar engines.
  - Per-row accumulator columns are the only device output; the cheap
    affine decode (tail corrections, a = N/L, b = R / max(2R - S + L*t,
    1e-8)) and final feature concatenation happen on the host.
"""

import math
import sys

import numpy as np

sys.path.insert(0, "/opt/trn_rl_repo")

C_IN, SEQ_LEN = 9, 2048
KERNEL_SIZE = 9
NUM_KERNELS = 84
B = 32
N_CORES = 8
B_CORE = B // N_CORES
PAD = 1020  # max padding over all dilations
LP = SEQ_LEN + 2 * PAD  # 4088
ZERO_W = 512  # width of the zeroed rhs region
NEG_BIG = -1.0e30

FP32_DILS = (24, 25)  # tiny valid regions: keep full fp32 precision
DVE_COST = 2.35  # us per [128, 2048] fused pass (measured)
ACT_COST = 2.20


def _config():
    nf_total = 10000 // 2 // NUM_KERNELS * NUM_KERNELS
    nfpk = nf_total // NUM_KERNELS
    true_max = min(nfpk, 32)
    multiplier = nfpk / true_max
    max_exp = np.log2((SEQ_LEN - 1) / (KERNEL_SIZE - 1))
    dilations, counts = np.unique(
        np.logspace(0, max_exp, true_max, base=2).astype(np.int32),
        return_counts=True)
    nfpd = (counts * multiplier).astype(np.int32)
    rem = nfpk - nfpd.sum()
    i = 0
    while rem > 0:
        nfpd[i] += 1
        rem -= 1
        i = (i + 1) % len(nfpd)
    paddings = [(KERNEL_SIZE - 1) * int(d) // 2 for d in dilations]
    return [int(d) for d in dilations], paddings, [int(n) for n in nfpd]


DILS, PADS, NFPD = _config()
ND = len(DILS)


class Plan:
    """Static schedule: pass columns, engine assignment, decode constants."""

    def __init__(self):
        self.passes = []  # list of dicts, one per (dil, type, f) pass family
        dve_load = 0.0
        act_load = 0.0
        for i, (d, p, nf) in enumerate(zip(DILS, PADS, NFPD)):
            fams = [("cnt", f) for f in range(nf)] + \
                   [("rel", f) for f in range(nf)] + [("sum", 0)]
            for typ, f in fams:
                if dve_load <= act_load:
                    eng, cost = "dve", DVE_COST
                else:
                    eng, cost = "act", ACT_COST
                if eng == "dve":
                    dve_load += 4 * cost
                else:
                    act_load += 4 * cost
                self.passes.append(dict(dil=i, typ=typ, f=f, eng=eng))
        # per-engine accumulator column indexing: each pass family uses
        # B_CORE consecutive columns (one per sample tile)
        ncol = {"dve": 0, "act": 0}
        thrcol = 0
        for pa in self.passes:
            pa["acc0"] = ncol[pa["eng"]]
            ncol[pa["eng"]] += B_CORE
            pa["thr"] = thrcol
            thrcol += 1
        self.ncol_dve = ncol["dve"]
        self.ncol_act = ncol["act"]
        self.nthr = thrcol
        self.off = np.concatenate([[0], np.cumsum(NFPD)]).astype(int)

    def halves(self, i):
        p1 = i % 2
        h0 = list(range(p1, NUM_KERNELS, 2))
        h1 = list(range(1 - p1, NUM_KERNELS, 2))
        return h0, h1

    def build_thresholds(self, biases):
        """thr[128, nthr] per-partition scalar operand for every pass family."""
        thr = np.zeros((128, self.nthr), np.float32)
        for pa in self.passes:
            i, typ, f, eng = pa["dil"], pa["typ"], pa["f"], pa["eng"]
            h0, h1 = self.halves(i)
            col = np.zeros(128, np.float32)
            if typ == "sum":
                col[:] = NEG_BIG if eng == "dve" else 0.0
            else:
                t0 = biases[h0, self.off[i] + f].astype(np.float32)
                t1 = biases[h1, self.off[i] + f].astype(np.float32)
                if eng == "dve":
                    col[0:42], col[64:106] = t0, t1
                elif typ == "cnt":
                    # ACT Sign pass: bias = -t', t' nudged off any fp32 value
                    # equal to a representable C so sign() never hits 0.
                    # C is fp32 here; nudge by ~1ulp of the threshold scale.
                    tp0 = np.nextafter(t0, np.float32(np.inf)).astype(np.float32)
                    tp1 = np.nextafter(t1, np.float32(np.inf)).astype(np.float32)
                    col[0:42], col[64:106] = -tp0, -tp1
                else:
                    col[0:42], col[64:106] = -t0, -t1
            thr[:, pa["thr"]] = col
        return thr

    def decode(self, acc_dve, acc_act, biases):
        """acc_*: [ncores, 128, ncol_*] fp32 -> output [B, 9912] fp32."""
        biases = np.asarray(biases, np.float64)
        nunits = 2  # h0 rows 0..41, h1 rows 64..105
        # gather per (core, b, dil): N[84, nf], R[84, nf], S[84]
        out = np.zeros((B, sum(168 * nf for nf in NFPD)), np.float64)
        # feature block offsets per dilation
        blk_off = np.concatenate([[0], np.cumsum([168 * nf for nf in NFPD])]).astype(int)
        for pa in self.passes:
            pa["_acc"] = (acc_dve if pa["eng"] == "dve" else acc_act)[
                :, :, pa["acc0"]:pa["acc0"] + B_CORE].astype(np.float64)
        N = {}
        R = {}
        S = {}
        for pa in self.passes:
            i, typ, f, eng = pa["dil"], pa["typ"], pa["f"], pa["eng"]
            d, p, nf = DILS[i], PADS[i], NFPD[i]
            h0, h1 = self.halves(i)
            V = SEQ_LEN - 2 * p          # valid length for h1
            T2 = SEQ_LEN - V             # zero-tail length for h1
            a = pa["_acc"]               # [ncores, 128, B_CORE]
            r0 = a[:, 0:42, :]           # h0 rows
            r1 = a[:, 64:106, :]         # h1 rows
            t0 = biases[h0, self.off[i] + f][None, :, None]
            t1 = biases[h1, self.off[i] + f][None, :, None]
            if typ == "cnt":
                if eng == "dve":
                    n0 = r0
                    n1 = r1 - T2 * (t1 < 0)
                else:
                    n0 = (r0 + SEQ_LEN) / 2
                    sv = r1 + T2 * np.sign(t1)
                    n1 = (sv + V) / 2
                N[(i, f)] = (n0, n1)
            elif typ == "rel":
                if eng == "dve":
                    rr0 = r0 - SEQ_LEN * t0
                    rr1 = r1 - T2 * np.maximum(t1, 0) - V * t1
                else:
                    rr0 = r0
                    rr1 = r1 - T2 * np.maximum(-t1, 0)
                R[(i, f)] = (rr0, rr1)
            else:
                S[i] = (r0, r1)
        for i, (d, p, nf) in enumerate(zip(DILS, PADS, NFPD)):
            h0, h1 = self.halves(i)
            V = SEQ_LEN - 2 * p
            s0, s1 = S[i]
            for f in range(nf):
                t0 = biases[h0, self.off[i] + f][None, :, None]
                t1 = biases[h1, self.off[i] + f][None, :, None]
                n0, n1 = N[(i, f)]
                r0, r1 = R[(i, f)]
                a0 = n0 / SEQ_LEN
                a1 = n1 / V
                d0 = s0 - SEQ_LEN * t0
                d1 = s1 - V * t1
                b0 = r0 / np.maximum(2 * r0 - d0, 1e-8)
                b1 = r1 / np.maximum(2 * r1 - d1, 1e-8)
                # output layout per dil: [a_h0(42,nf) | b_h0 | a_h1 | b_h1],
                # feature index within block = jj*nf + f
                base = blk_off[i]
                for core in range(N_CORES):
                    for bb in range(B_CORE):
                        gb = core * B_CORE + bb
                        out[gb, base + f + nf * np.arange(42)] = a0[core, :, bb]
                        out[gb, base + 42 * nf + f + nf * np.arange(42)] = b0[core, :, bb]
                        out[gb, base + 84 * nf + f + nf * np.arange(42)] = a1[core, :, bb]
                        out[gb, base + 126 * nf + f + nf * np.arange(42)] = b1[core, :, bb]
        return out.astype(np.float32)


PLAN = Plan()
_NC_CACHE = {}


def _build_nc():
    import concourse.bacc as bacc
    import concourse.tile as tile
    from concourse import mybir

    nc = bacc.Bacc()
    x_pad = nc.dram_tensor("x_pad", [C_IN, B_CORE, LP], mybir.dt.bfloat16,
                           kind="ExternalInput")
    x_pad32 = nc.dram_tensor("x_pad32", [C_IN, B_CORE, LP], mybir.dt.float32,
                             kind="ExternalInput")
    w2 = nc.dram_tensor("w2", [ND, 2, 81, 64], mybir.dt.bfloat16,
                        kind="ExternalInput")
    w232 = nc.dram_tensor("w232", [len(FP32_DILS), 2, 81, 64], mybir.dt.float32,
                          kind="ExternalInput")
    thr_in = nc.dram_tensor("thr_in", [128, PLAN.nthr], mybir.dt.float32,
                            kind="ExternalInput")
    acc_dve_out = nc.dram_tensor("acc_dve", [128, PLAN.ncol_dve],
                                 mybir.dt.float32, kind="ExternalOutput")
    acc_act_out = nc.dram_tensor("acc_act", [128, PLAN.ncol_act],
                                 mybir.dt.float32, kind="ExternalOutput")

    WMAX = SEQ_LEN                      # per-sample slab width
    XCOLS = B_CORE * WMAX + ZERO_W

    with tile.TileContext(nc) as tc:
        with tc.tile_pool(name="sb", bufs=1) as sb, \
             tc.tile_pool(name="ps", bufs=2, space="PSUM") as ps:
            w2_sb = sb.tile([81, ND, 2, 64], mybir.dt.bfloat16, tag="w2")
            w232_sb = sb.tile([81, len(FP32_DILS), 2, 64], mybir.dt.float32, tag="w232")
            thr = sb.tile([128, PLAN.nthr], mybir.dt.float32, tag="thr")
            acc_dve = sb.tile([128, PLAN.ncol_dve], mybir.dt.float32, tag="accd")
            acc_act = sb.tile([128, PLAN.ncol_act], mybir.dt.float32, tag="acca")
            junk_d = sb.tile([128, SEQ_LEN], mybir.dt.bfloat16, tag="junkd")
            junk_a = sb.tile([128, SEQ_LEN], mybir.dt.bfloat16, tag="junka")
            xmat = []
            for k in range(2):
                xmk = sb.tile([81, XCOLS], mybir.dt.bfloat16, tag=f"xmat{k}")
                xmat.append(xmk)
            xmat32 = sb.tile([81, XCOLS], mybir.dt.float32, tag="xmat32")

            nc.sync.dma_start(out=w2_sb,
                              in_=w2[:, :, :, :].transpose([2, 0, 1, 3]))
            nc.sync.dma_start(out=w232_sb,
                              in_=w232[:, :, :, :].transpose([2, 0, 1, 3]))
            nc.sync.dma_start(out=thr, in_=thr_in[:, :])
            for k in range(2):
                nc.vector.memset(xmat[k][:, B_CORE * WMAX:], 0.0)
            nc.vector.memset(xmat32[:, B_CORE * WMAX:], 0.0)

            for i, (d, p, nf) in enumerate(zip(DILS, PADS, NFPD)):
                W = SEQ_LEN              # per-sample slab width
                is32 = i in FP32_DILS
                xm = xmat32 if is32 else xmat[i % 2]
                xsrc = x_pad32 if is32 else x_pad
                # gather the shifted slab straight from DRAM:
                # xm[(t*9+c), b*W + l] = x_pad[c, b, PAD - 4d + d*t + l]
                import concourse.bass as bass
                full = xsrc[:, :, :]
                for t in range(9):
                    src_ap = bass.AP(
                        tensor=full.tensor,
                        offset=PAD - 4 * d + d * t,
                        ap=[[B_CORE * LP, 9], [LP, B_CORE], [1, W]],
                    )
                    dst_ap = bass.AP(
                        tensor=xm.tensor,
                        offset=xm.offset + t * 9 * XCOLS,
                        ap=[[XCOLS, 9], [W, B_CORE], [1, W]],
                    )
                    nc.sync.dma_start(out=dst_ap, in_=src_ap)

                if is32:
                    lhs0 = w232_sb[:, FP32_DILS.index(i), 0, :]
                    lhs1 = w232_sb[:, FP32_DILS.index(i), 1, :]
                else:
                    lhs0 = w2_sb[:, i, 0, :]
                    lhs1 = w2_sb[:, i, 1, :]
                V = SEQ_LEN - 2 * p
                kv = V // 512
                for bb in range(B_CORE):
                    pt = ps.tile([128, SEQ_LEN], mybir.dt.float32, tag="pt")
                    x0 = bb * W
                    # h0: full window [0, 2048)
                    for k in range(4):
                        nc.tensor.matmul(pt[0:64, 512 * k:512 * (k + 1)],
                                         lhs0, xm[:, x0 + 512 * k: x0 + 512 * (k + 1)],
                                         start=True, stop=True)
                    # h1: window [p, p+2048); tail beyond valid uses zero rhs
                    for k in range(kv):
                        nc.tensor.matmul(pt[64:128, 512 * k:512 * (k + 1)],
                                         lhs1,
                                         xm[:, x0 + p + 512 * k: x0 + p + 512 * (k + 1)],
                                         start=True, stop=True)
                    rem = V - 512 * kv
                    if rem > 0:
                        nc.tensor.matmul(pt[64:128, 512 * kv:512 * kv + rem],
                                         lhs1, xm[:, x0 + p + 512 * kv: x0 + 2048 - p],
                                         start=True, stop=True)
                    zoff = B_CORE * WMAX
                    zstart = V
                    while zstart < SEQ_LEN:
                        zlen = min(512, SEQ_LEN - zstart,
                                   512 * (zstart // 512 + 1) - zstart)
                        nc.tensor.matmul(pt[64:128, zstart:zstart + zlen],
                                         lhs1, xm[:, zoff:zoff + zlen],
                                         start=True, stop=True)
                        zstart += zlen

                    # fused reduce passes over the finished tile
                    for pa in PLAN.passes:
                        if pa["dil"] != i:
                            continue
                        tcol = thr[:, pa["thr"]:pa["thr"] + 1]
                        if pa["eng"] == "dve":
                            acol = acc_dve[:, pa["acc0"] + bb: pa["acc0"] + bb + 1]
                            op0 = (mybir.AluOpType.is_gt if pa["typ"] == "cnt"
                                   else mybir.AluOpType.max)
                            nc.vector.tensor_scalar(
                                out=junk_d, in0=pt[:, :], scalar1=tcol,
                                scalar2=None, op0=op0,
                                op1=mybir.AluOpType.add, accum_out=acol)
                        else:
                            acol = acc_act[:, pa["acc0"] + bb: pa["acc0"] + bb + 1]
                            if pa["typ"] == "cnt":
                                nc.scalar.activation(
                                    out=junk_a, in_=pt[:, :],
                                    func=mybir.ActivationFunctionType.Sign,
                                    bias=tcol, scale=1.0, accum_out=acol)
                            elif pa["typ"] == "rel":
                                nc.scalar.activation(
                                    out=junk_a, in_=pt[:, :],
                                    func=mybir.ActivationFunctionType.Relu,
                                    bias=tcol, scale=1.0, accum_out=acol)
                            else:
                                nc.scalar.activation(
                                    out=junk_a, in_=pt[:, :],
                                    func=mybir.ActivationFunctionType.Copy,
                                    bias=0.0, scale=1.0, accum_out=acol)

            nc.sync.dma_start(out=acc_dve_out[:, :], in_=acc_dve)
            nc.sync.dma_start(out=acc_act_out[:, :], in_=acc_act)
    nc.compile()
    return nc


def _host_prep(x, kernels, channel_combinations):
    x = np.asarray(x, np.float32)
    kernels = np.asarray(kernels, np.float32)
    cc = np.asarray(channel_combinations, np.float32)
    import ml_dtypes
    x_pad = np.zeros((B, C_IN, LP), np.float32)
    x_pad[:, :, PAD:PAD + SEQ_LEN] = x
    # W2[i, h, t*9+c, jj] = cc[i, c, j] * kernels[c*84+j, 0, t], j = halves
    kern = kernels.reshape(C_IN, NUM_KERNELS, KERNEL_SIZE)  # [c, j, t]
    w2 = np.zeros((ND, 2, 81, 64), np.float32)
    for i in range(ND):
        full = np.einsum("cj,cjt->tcj", cc[i], kern)  # [t, c, j]
        full = full.reshape(81, NUM_KERNELS)
        h0, h1 = PLAN.halves(i)
        w2[i, 0, :, :42] = full[:, h0]
        w2[i, 1, :, :42] = full[:, h1]
    return x_pad, w2


def _make_in_maps(x_pad, w2, thr, cores):
    import ml_dtypes
    w2bf = w2.astype(ml_dtypes.bfloat16)
    w2f = np.ascontiguousarray(w2[list(FP32_DILS)])
    in_maps = []
    for core in cores:
        xs = np.ascontiguousarray(
            x_pad[core * B_CORE:(core + 1) * B_CORE].transpose(1, 0, 2))
        in_maps.append({
            "x_pad": xs.astype(ml_dtypes.bfloat16),
            "x_pad32": xs,
            "w2": w2bf,
            "w232": w2f,
            "thr_in": thr,
        })
    return in_maps


def kernel(x, kernels, channel_combinations, biases, _run_cores=None):
    from concourse.bass_utils import run_bass_kernel_spmd

    x_pad, w2 = _host_prep(x, kernels, channel_combinations)
    thr = PLAN.build_thresholds(np.asarray(biases, np.float32))

    if "nc" not in _NC_CACHE:
        _NC_CACHE["nc"] = _build_nc()
    nc = _NC_CACHE["nc"]

    cores = list(range(N_CORES)) if _run_cores is None else _run_cores
    in_maps = _make_in_maps(x_pad, w2, thr, cores)
    res = run_bass_kernel_spmd(nc, in_maps, core_ids=cores)
    acc_dve = np.stack([r["acc_dve"] for r in res.results])
    acc_act = np.stack([r["acc_act"] for r in res.results])
    if _run_cores is not None:  # partial run (debug): tile results
        reps = N_CORES // len(cores)
        acc_dve = np.concatenate([acc_dve] * reps)
        acc_act = np.concatenate([acc_act] * reps)
    return PLAN.decode(acc_dve, acc_act, biases)


def sim_accums(x, kernels, channel_combinations, biases):
    """Numpy simulation of the device accumulators (for decode validation)."""
    import ml_dtypes
    x_pad, w2 = _host_prep(x, kernels, channel_combinations)
    x_bf = x_pad.astype(ml_dtypes.bfloat16).astype(np.float32)
    w_bf = w2.astype(ml_dtypes.bfloat16).astype(np.float32)
    acc_dve = np.zeros((N_CORES, 128, PLAN.ncol_dve), np.float32)
    acc_act = np.zeros((N_CORES, 128, PLAN.ncol_act), np.float32)
    thr = PLAN.build_thresholds(np.asarray(biases, np.float32))
    for core in range(N_CORES):
        for i, (d, p, nf) in enumerate(zip(DILS, PADS, NFPD)):
            is32 = i in FP32_DILS
            xs = (x_pad if is32 else x_bf)[core * B_CORE:(core + 1) * B_CORE]
            wse = (w2 if is32 else w_bf)
            W = SEQ_LEN
            V = SEQ_LEN - 2 * p
            # Xslab[k=(t,c), b, l]
            xsl = np.zeros((81, B_CORE, W), np.float32)
            for t in range(9):
                for c in range(9):
                    o = PAD - 4 * d + d * t
                    xsl[t * 9 + c] = xs[:, c, o:o + W]
            for bb in range(B_CORE):
                C = np.zeros((128, SEQ_LEN), np.float32)
                C[0:64] = wse[i, 0].T @ xsl[:, bb, :SEQ_LEN]
                C[64:128, :V] = wse[i, 1].T @ xsl[:, bb, p:SEQ_LEN - p]
                for pa in PLAN.passes:
                    if pa["dil"] != i:
                        continue
                    tc_ = thr[:, pa["thr"]][:, None]
                    if pa["eng"] == "dve":
                        if pa["typ"] == "cnt":
                            v = (C > tc_).sum(1, dtype=np.float32)
                        else:
                            v = np.maximum(C, tc_).sum(1, dtype=np.float64).astype(np.float32)
                        acc_dve[core, :, pa["acc0"] + bb] = v
                    else:
                        if pa["typ"] == "cnt":
                            v = np.sign(C + tc_).sum(1).astype(np.float32)
                        elif pa["typ"] == "rel":
                            v = np.maximum(C + tc_, 0).sum(1, dtype=np.float64).astype(np.float32)
                        else:
                            v = C.sum(1, dtype=np.float64).astype(np.float32)
                        acc_act[core, :, pa["acc0"] + bb] = v
    return acc_dve, acc_act
